# revision 17
# baseline (speedup 1.0000x reference)
"""DCT-attention kernel for Trainium2 (8 NeuronCores, batch data-parallel).

The reference applies an orthonormal DCT-II followed immediately by its
inverse over the T axis -- mathematically the identity -- then dense
self-attention over the C axis with 1/sqrt(32) scaling.  So the kernel
computes, for each of the B*T = 2048 independent [C=128, W=128] tiles A:

    O = softmax(A @ A.T / sqrt(32)) @ A

v2 design -- everything is shaped to make the DMA byte-roofline the only
bottleneck (the v1 kernel was DMA-descriptor-bound: ~99k packets of
256-512B at ~23ns each):

  * The host uploads two fp16 copies of the input per core:
      Xa [C=128, T*129]  -- A tiles (partition=c) with a ones column
                            appended to every tile,
      Xt [W=128, T*128]  -- A^T tiles (partition=w).
    Per-partition runs are huge and contiguous, so every DMA moves
    ~1MB in maximal packets.  The pre-transposed Xt removes the xbar
    DMA transpose entirely; the ones column makes MM2 produce the
    softmax row sums for free (col 128 of each [128,129] output).
  * MM1: S = At.T @ At (fp16 x fp16, N=128).  exp on ACT in 8-tile
    batches (FD=1024, 2 PSUM banks) -> E bf16 in SBUF.  E is symmetric
    so it feeds MM2 as the stationary operand unchanged.
  * MM2: [O | r] = E.T @ [A | 1] (bf16 x fp16, N=129), 3 tiles per
    PSUM bank (129*3*4B < 2KB).
  * Row-sum reciprocals batched 12 tiles per DVE op; eviction
    PSUM->SBUF is a single fused tensor_tensor multiply with a
    stride-0 broadcast of rinv (3 tiles per op), writing fp16.
  * Store: fp16 [C, T*128] in 32-tile (1MB) chunks on the scalar-engine
    HWDGE ring; the host transposes back and upcasts.

Sharding: batch axis B=8 across the 8 cores, 256 tiles per core.
"""

from contextlib import ExitStack

import numpy as np

import concourse.bass as bass
import concourse.mybir as mybir
import concourse.tile as tile
from concourse import bacc
from concourse.bass_utils import run_bass_kernel_spmd

B, T, C, W = 8, 256, 128, 128
N_CORES = 8
SCALE = float(1.0 / np.sqrt(32.0))
F32 = mybir.dt.float32
F16 = mybir.dt.float16
BF16 = mybir.dt.bfloat16

LOAD_CHUNK = 32      # tiles per load DMA (~1MB each)
STORE_CHUNK = 32     # tiles per store DMA
EXP_GROUP = 8        # tiles per ACT exp call (2 PSUM banks, FD=1024)
S_SLOTS = 2          # MM1 output slots (2 banks each)
PACK = 3             # MM2 outputs per PSUM bank (3*129*4B <= 2KB)
REC_PACKS = 4        # o-ring size in packs (= #o-banks)
EVICT_PACKS = 2      # packs per rec/evict span (adjacent banks)
E_TILES = 48         # E ring size in tiles
A_LAG = 20           # MM1 runs this many tiles ahead of MM2
EXP_HALF = 8         # tiles per ACT exp call (a full s_ring slot)
O_TILES = 96         # output SBUF ring size in tiles (divisible by PACK and STORE_CHUNK)


def build_nc() -> bass.Bass:
    nc = bacc.Bacc("TRN2", debug=False)
    xa = nc.dram_tensor("Xa", [128, T * 129], F16, kind="ExternalInput").ap()
    xt = nc.dram_tensor("Xt", [128, T * 128], F16, kind="ExternalInput").ap()
    y = nc.dram_tensor("out", [128, T * 128], F16, kind="ExternalOutput").ap()

    with tile.TileContext(nc) as tc, ExitStack() as ctx:
        sb = ctx.enter_context(tc.tile_pool(name="sb", bufs=1))
        ps = ctx.enter_context(tc.tile_pool(name="ps", bufs=1, space="PSUM"))

        xa_sb = sb.tile([128, T * 129], F16)
        xt_sb = sb.tile([128, T * 128], F16)
        e_sb = sb.tile([128, E_TILES * 128], BF16)
        o_sb = sb.tile([128, O_TILES * 128], F16)
        rinv = sb.tile([128, T], F32)

        # separate tile objects per PSUM bank pair — coarse dependency
        # tracking on one big PSUM tile was observed to serialize spans
        s_rings = [
            ps.tile([128, EXP_GROUP * 128], F32, name=f"s_ring{i}")
            for i in range(S_SLOTS)
        ]
        o_rings = [
            ps.tile([128, EVICT_PACKS * 512], F32, name=f"o_ring{i}")
            for i in range(REC_PACKS // EVICT_PACKS)
        ]

        # ---- loads: interleave the two streams in big chunks ----
        for k in range(T // LOAD_CHUNK):
            nc.sync.dma_start(
                xt_sb[:, k * LOAD_CHUNK * 128 : (k + 1) * LOAD_CHUNK * 128],
                xt[:, k * LOAD_CHUNK * 128 : (k + 1) * LOAD_CHUNK * 128],
            )
            nc.sync.dma_start(
                xa_sb[:, k * LOAD_CHUNK * 129 : (k + 1) * LOAD_CHUNK * 129],
                xa[:, k * LOAD_CHUNK * 129 : (k + 1) * LOAD_CHUNK * 129],
            )

        # pack p covers tiles [3p, 3p+3) except the tail pack (tile 255).
        n_packs = (T + PACK - 1) // PACK

        def emit_mm1(t):
            s_t = s_rings[(t // EXP_GROUP) % S_SLOTS]
            j = t % EXP_GROUP
            at = xt_sb[:, t * 128 : (t + 1) * 128]
            nc.tensor.matmul(
                s_t[:, j * 128 : (j + 1) * 128],
                lhsT=at,
                rhs=at,
                start=True,
                stop=True,
            )

        def emit_exp_half(h):
            """exp for tiles [h*EXP_HALF, (h+1)*EXP_HALF)."""
            t0 = h * EXP_HALF
            s_t = s_rings[(t0 // EXP_GROUP) % S_SLOTS]
            off = (t0 % EXP_GROUP) * 128
            eo = (t0 % E_TILES) * 128
            nc.scalar.activation(
                e_sb[:, eo : eo + EXP_HALF * 128],
                s_t[:, off : off + EXP_HALF * 128],
                mybir.ActivationFunctionType.Exp,
                scale=SCALE,
            )

        def emit_mm2(t):
            p = t // PACK
            o_t = o_rings[(p // EVICT_PACKS) % len(o_rings)]
            off = (p % EVICT_PACKS) * 512 + (t % PACK) * 129
            e = e_sb[:, (t % E_TILES) * 128 : (t % E_TILES + 1) * 128]
            nc.tensor.matmul(
                o_t[:, off : off + 129],
                lhsT=e,
                rhs=xa_sb[:, t * 129 : (t + 1) * 129],
                start=True,
                stop=True,
            )

        def emit_rec(p0, p1):
            """reciprocal of row sums for packs [p0, p1) (one bank pair)."""
            t0 = p0 * PACK
            t1 = min(p1 * PACK, T)
            full = t1 - t0 == (p1 - p0) * PACK
            o_t = o_rings[(p0 // EVICT_PACKS) % len(o_rings)]
            if full:
                # r columns of the span's banks in one strided AP
                r_ap = o_t[:, : (p1 - p0) * 512].rearrange(
                    "p (k x) -> p k x", k=p1 - p0
                )[:, :, 128:512:129]
                nc.vector.reciprocal(rinv[:, t0:t1], r_ap)
            else:
                for p in range(p0, p1):
                    a = p * PACK
                    b = min(a + PACK, T)
                    pb = (p % EVICT_PACKS) * 512
                    stop = pb + 128 + (b - a - 1) * 129 + 1
                    r_ap = o_t[:, pb + 128 : stop : 129]
                    nc.vector.reciprocal(rinv[:, a:b], r_ap)

        def emit_evict(p0, p1):
            """scale + evict packs [p0, p1) (one bank pair) in one DVE op."""
            a = p0 * PACK
            b = min(p1 * PACK, T)
            o_t = o_rings[(p0 // EVICT_PACKS) % len(o_rings)]
            if b - a == (p1 - p0) * PACK:
                # uniform span: AP [128, npacks, PACK, 129->128]
                src = o_t[:, : (p1 - p0) * 512].rearrange(
                    "p (k x) -> p k x", k=p1 - p0
                )[:, :, : PACK * 129].rearrange(
                    "p k (t c) -> p k t c", t=PACK
                )[:, :, :, :128]
                sc = (
                    rinv[:, a:b]
                    .rearrange("p (k t) -> p k t", k=p1 - p0)
                    .unsqueeze(3)
                    .broadcast_to([128, p1 - p0, PACK, 128])
                )
                dst = o_sb[
                    :, (a % O_TILES) * 128 : (a % O_TILES) * 128 + (b - a) * 128
                ].rearrange("p (k t c) -> p k t c", k=p1 - p0, t=PACK)
                nc.vector.tensor_tensor(dst, src, sc, op=mybir.AluOpType.mult)
            else:
                for p in range(p0, p1):
                    pa = p * PACK
                    pb = min(pa + PACK, T)
                    n = pb - pa
                    pbase = (p % EVICT_PACKS) * 512
                    src = o_t[:, pbase : pbase + n * 129].rearrange(
                        "p (t c) -> p t c", t=n
                    )[:, :, :128]
                    sc = rinv[:, pa:pb].unsqueeze(2).broadcast_to([128, n, 128])
                    dst = o_sb[
                        :, (pa % O_TILES) * 128 : (pa % O_TILES) * 128 + n * 128
                    ].rearrange("p (t c) -> p t c", t=n)
                    nc.vector.tensor_tensor(dst, src, sc, op=mybir.AluOpType.mult)

        def emit_store(k):
            t0 = k * STORE_CHUNK
            nc.scalar.dma_start(
                y[:, t0 * 128 : (t0 + STORE_CHUNK) * 128],
                o_sb[
                    :,
                    (t0 % O_TILES) * 128 : (t0 % O_TILES) * 128
                    + STORE_CHUNK * 128,
                ],
            )

        # ---- main pipeline (fine-grained interleave) ----
        # PE stream per tile-step t: [MM2(t), MM1(t+A_LAG)].  Every
        # instruction's producers ran ~A_LAG tiles earlier, so nothing
        # stalls at the PE FIFO head (stalled heads block everything
        # behind and re-throttle the PE clock).  exp is split into two
        # FD=512 halves so MM1s overwriting an s_ring slot only wait on
        # the half that covers their columns.  rec+evict fire at every
        # 6-tile span boundary and MUST be emitted before any MM2 of the
        # next span into the same PSUM bank pair (program order is what
        # the Tile dependency tracker sees).
        rec_next = 0          # next pack to rec+evict
        store_next = 0        # next store chunk

        for step in range(-A_LAG, T):
            tm1 = step + A_LAG
            if tm1 < T:
                emit_mm1(tm1)
                if tm1 % EXP_HALF == EXP_HALF - 1:
                    emit_exp_half(tm1 // EXP_HALF)
            t = step
            if t < 0:
                continue
            emit_mm2(t)
            if (t + 1) % (EVICT_PACKS * PACK) == 0 or t + 1 == T:
                p1 = (t + 1 + PACK - 1) // PACK
                emit_rec(rec_next, p1)
                emit_evict(rec_next, p1)
                rec_next = p1
                while (store_next + 1) * STORE_CHUNK <= rec_next * PACK:
                    emit_store(store_next)
                    store_next += 1
        assert rec_next == n_packs
        assert store_next * STORE_CHUNK == T

    nc.compile()
    return nc


_NC_CACHE: dict[str, bass.Bass] = {}


def _get_nc() -> bass.Bass:
    if "nc" not in _NC_CACHE:
        _NC_CACHE["nc"] = build_nc()
    return _NC_CACHE["nc"]


def _prep_core(Xi: np.ndarray) -> dict[str, np.ndarray]:
    """Xi: [T, C, W] fp32 -> host-side fp16 layouts."""
    xa = np.empty((C, T, 129), dtype=np.float16)
    xa[:, :, :128] = Xi.transpose(1, 0, 2)
    xa[:, :, 128] = 1.0
    xt = np.ascontiguousarray(Xi.transpose(2, 0, 1)).astype(np.float16)
    return {
        "Xa": xa.reshape(128, T * 129),
        "Xt": xt.reshape(128, T * 128),
    }


def run(X: np.ndarray, **spmd_kwargs):
    """Shard over batch, run on 8 cores, gather.  Returns (output, results)."""
    assert X.shape == (B, T, C, W), X.shape
    nc = _get_nc()
    in_maps = [_prep_core(X[i]) for i in range(N_CORES)]
    res = run_bass_kernel_spmd(nc, in_maps, list(range(N_CORES)), **spmd_kwargs)
    out = np.stack(
        [
            res.results[i]["out"]
            .reshape(C, T, W)
            .transpose(1, 0, 2)
            .astype(np.float32)
            for i in range(N_CORES)
        ],
        axis=0,
    )
    return out, res


def kernel(X: np.ndarray) -> np.ndarray:
    out, _ = run(np.asarray(X, dtype=np.float32))
    return out


# revision 18
# speedup vs baseline: 1.0125x; 1.0125x over previous
"""DCT-attention kernel for Trainium2 (8 NeuronCores, batch data-parallel).

The reference applies an orthonormal DCT-II followed immediately by its
inverse over the T axis -- mathematically the identity -- then dense
self-attention over the C axis with 1/sqrt(32) scaling.  So the kernel
computes, for each of the B*T = 2048 independent [C=128, W=128] tiles A:

    O = softmax(A @ A.T / sqrt(32)) @ A

v2 design -- everything is shaped to make the DMA byte-roofline the only
bottleneck (the v1 kernel was DMA-descriptor-bound: ~99k packets of
256-512B at ~23ns each):

  * The host uploads two fp16 copies of the input per core:
      Xa [C=128, T*129]  -- A tiles (partition=c) with a ones column
                            appended to every tile,
      Xt [W=128, T*128]  -- A^T tiles (partition=w).
    Per-partition runs are huge and contiguous, so every DMA moves
    ~1MB in maximal packets.  The pre-transposed Xt removes the xbar
    DMA transpose entirely; the ones column makes MM2 produce the
    softmax row sums for free (col 128 of each [128,129] output).
  * MM1: S = At.T @ At (fp16 x fp16, N=128).  exp on ACT in 8-tile
    batches (FD=1024, 2 PSUM banks) -> E bf16 in SBUF.  E is symmetric
    so it feeds MM2 as the stationary operand unchanged.
  * MM2: [O | r] = E.T @ [A | 1] (bf16 x fp16, N=129), 3 tiles per
    PSUM bank (129*3*4B < 2KB).
  * Row-sum reciprocals batched 12 tiles per DVE op; eviction
    PSUM->SBUF is a single fused tensor_tensor multiply with a
    stride-0 broadcast of rinv (3 tiles per op), writing fp16.
  * Store: fp16 [C, T*128] in 32-tile (1MB) chunks on the scalar-engine
    HWDGE ring; the host transposes back and upcasts.

Sharding: batch axis B=8 across the 8 cores, 256 tiles per core.
"""

from contextlib import ExitStack

import numpy as np

import concourse.bass as bass
import concourse.mybir as mybir
import concourse.tile as tile
from concourse import bacc
from concourse.bass_utils import run_bass_kernel_spmd

B, T, C, W = 8, 256, 128, 128
N_CORES = 8
SCALE = float(1.0 / np.sqrt(32.0))
F32 = mybir.dt.float32
F16 = mybir.dt.float16
BF16 = mybir.dt.bfloat16

LOAD_CHUNK = 32      # tiles per load DMA (~1MB each)
STORE_CHUNK = 32     # tiles per store DMA
EXP_GROUP = 8        # tiles per ACT exp call (2 PSUM banks, FD=1024)
S_SLOTS = 2          # MM1 output slots (2 banks each)
PACK = 3             # MM2 outputs per PSUM bank (3*129*4B <= 2KB)
REC_PACKS = 4        # o-ring size in packs (= #o-banks)
EVICT_PACKS = 2      # packs per rec/evict span (adjacent banks)
E_TILES = 48         # E ring size in tiles
A_LAG = 20           # MM1 runs this many tiles ahead of MM2
EXP_HALF = 4         # tiles per ACT exp call (half an s_ring slot)
O_TILES = 96         # output SBUF ring size in tiles (divisible by PACK and STORE_CHUNK)


def build_nc() -> bass.Bass:
    nc = bacc.Bacc("TRN2", debug=False)
    xa = nc.dram_tensor("Xa", [128, T * 129], F16, kind="ExternalInput").ap()
    xt = nc.dram_tensor("Xt", [128, T * 128], F16, kind="ExternalInput").ap()
    y = nc.dram_tensor("out", [128, T * 128], F16, kind="ExternalOutput").ap()

    with tile.TileContext(nc) as tc, ExitStack() as ctx:
        sb = ctx.enter_context(tc.tile_pool(name="sb", bufs=1))
        ps = ctx.enter_context(tc.tile_pool(name="ps", bufs=1, space="PSUM"))

        xa_sb = sb.tile([128, T * 129], F16)
        xt_sb = sb.tile([128, T * 128], F16)
        e_sb = sb.tile([128, E_TILES * 128], BF16)
        o_sb = sb.tile([128, O_TILES * 128], F16)
        rinv = sb.tile([128, T], F32)

        # separate tile objects per PSUM bank pair — coarse dependency
        # tracking on one big PSUM tile was observed to serialize spans
        s_rings = [
            ps.tile([128, EXP_GROUP * 128], F32, name=f"s_ring{i}")
            for i in range(S_SLOTS)
        ]
        o_rings = [
            ps.tile([128, EVICT_PACKS * 512], F32, name=f"o_ring{i}")
            for i in range(REC_PACKS // EVICT_PACKS)
        ]

        # ---- loads: interleave the two streams in big chunks ----
        for k in range(T // LOAD_CHUNK):
            nc.sync.dma_start(
                xt_sb[:, k * LOAD_CHUNK * 128 : (k + 1) * LOAD_CHUNK * 128],
                xt[:, k * LOAD_CHUNK * 128 : (k + 1) * LOAD_CHUNK * 128],
            )
            nc.sync.dma_start(
                xa_sb[:, k * LOAD_CHUNK * 129 : (k + 1) * LOAD_CHUNK * 129],
                xa[:, k * LOAD_CHUNK * 129 : (k + 1) * LOAD_CHUNK * 129],
            )

        # pack p covers tiles [3p, 3p+3) except the tail pack (tile 255).
        n_packs = (T + PACK - 1) // PACK

        def emit_mm1(t):
            s_t = s_rings[(t // EXP_GROUP) % S_SLOTS]
            j = t % EXP_GROUP
            at = xt_sb[:, t * 128 : (t + 1) * 128]
            nc.tensor.matmul(
                s_t[:, j * 128 : (j + 1) * 128],
                lhsT=at,
                rhs=at,
                start=True,
                stop=True,
            )

        def emit_exp_half(h):
            """exp for tiles [h*EXP_HALF, (h+1)*EXP_HALF)."""
            t0 = h * EXP_HALF
            s_t = s_rings[(t0 // EXP_GROUP) % S_SLOTS]
            off = (t0 % EXP_GROUP) * 128
            eo = (t0 % E_TILES) * 128
            nc.scalar.activation(
                e_sb[:, eo : eo + EXP_HALF * 128],
                s_t[:, off : off + EXP_HALF * 128],
                mybir.ActivationFunctionType.Exp,
                scale=SCALE,
            )

        def emit_mm2(t):
            p = t // PACK
            o_t = o_rings[(p // EVICT_PACKS) % len(o_rings)]
            off = (p % EVICT_PACKS) * 512 + (t % PACK) * 129
            e = e_sb[:, (t % E_TILES) * 128 : (t % E_TILES + 1) * 128]
            nc.tensor.matmul(
                o_t[:, off : off + 129],
                lhsT=e,
                rhs=xa_sb[:, t * 129 : (t + 1) * 129],
                start=True,
                stop=True,
            )

        def emit_rec(p0, p1):
            """reciprocal of row sums for packs [p0, p1) (one bank pair)."""
            t0 = p0 * PACK
            t1 = min(p1 * PACK, T)
            full = t1 - t0 == (p1 - p0) * PACK
            o_t = o_rings[(p0 // EVICT_PACKS) % len(o_rings)]
            if full:
                # r columns of the span's banks in one strided AP
                r_ap = o_t[:, : (p1 - p0) * 512].rearrange(
                    "p (k x) -> p k x", k=p1 - p0
                )[:, :, 128:512:129]
                nc.vector.reciprocal(rinv[:, t0:t1], r_ap)
            else:
                for p in range(p0, p1):
                    a = p * PACK
                    b = min(a + PACK, T)
                    pb = (p % EVICT_PACKS) * 512
                    stop = pb + 128 + (b - a - 1) * 129 + 1
                    r_ap = o_t[:, pb + 128 : stop : 129]
                    nc.vector.reciprocal(rinv[:, a:b], r_ap)

        def emit_evict(p0, p1):
            """scale + evict packs [p0, p1) (one bank pair) in one DVE op."""
            a = p0 * PACK
            b = min(p1 * PACK, T)
            o_t = o_rings[(p0 // EVICT_PACKS) % len(o_rings)]
            if b - a == (p1 - p0) * PACK:
                # uniform span: AP [128, npacks, PACK, 129->128]
                src = o_t[:, : (p1 - p0) * 512].rearrange(
                    "p (k x) -> p k x", k=p1 - p0
                )[:, :, : PACK * 129].rearrange(
                    "p k (t c) -> p k t c", t=PACK
                )[:, :, :, :128]
                sc = (
                    rinv[:, a:b]
                    .rearrange("p (k t) -> p k t", k=p1 - p0)
                    .unsqueeze(3)
                    .broadcast_to([128, p1 - p0, PACK, 128])
                )
                dst = o_sb[
                    :, (a % O_TILES) * 128 : (a % O_TILES) * 128 + (b - a) * 128
                ].rearrange("p (k t c) -> p k t c", k=p1 - p0, t=PACK)
                nc.vector.tensor_tensor(dst, src, sc, op=mybir.AluOpType.mult)
            else:
                for p in range(p0, p1):
                    pa = p * PACK
                    pb = min(pa + PACK, T)
                    n = pb - pa
                    pbase = (p % EVICT_PACKS) * 512
                    src = o_t[:, pbase : pbase + n * 129].rearrange(
                        "p (t c) -> p t c", t=n
                    )[:, :, :128]
                    sc = rinv[:, pa:pb].unsqueeze(2).broadcast_to([128, n, 128])
                    dst = o_sb[
                        :, (pa % O_TILES) * 128 : (pa % O_TILES) * 128 + n * 128
                    ].rearrange("p (t c) -> p t c", t=n)
                    nc.vector.tensor_tensor(dst, src, sc, op=mybir.AluOpType.mult)

        def emit_store(k):
            t0 = k * STORE_CHUNK
            nc.scalar.dma_start(
                y[:, t0 * 128 : (t0 + STORE_CHUNK) * 128],
                o_sb[
                    :,
                    (t0 % O_TILES) * 128 : (t0 % O_TILES) * 128
                    + STORE_CHUNK * 128,
                ],
            )

        # ---- main pipeline (fine-grained interleave) ----
        # PE stream per tile-step t: [MM2(t), MM1(t+A_LAG)].  Every
        # instruction's producers ran ~A_LAG tiles earlier, so nothing
        # stalls at the PE FIFO head (stalled heads block everything
        # behind and re-throttle the PE clock).  exp is split into two
        # FD=512 halves so MM1s overwriting an s_ring slot only wait on
        # the half that covers their columns.  rec+evict fire at every
        # 6-tile span boundary and MUST be emitted before any MM2 of the
        # next span into the same PSUM bank pair (program order is what
        # the Tile dependency tracker sees).
        rec_next = 0          # next pack to rec+evict
        store_next = 0        # next store chunk

        for step in range(-A_LAG, T):
            tm1 = step + A_LAG
            if tm1 < T:
                emit_mm1(tm1)
                if tm1 % EXP_HALF == EXP_HALF - 1:
                    emit_exp_half(tm1 // EXP_HALF)
            t = step
            if t < 0:
                continue
            emit_mm2(t)
            if (t + 1) % (EVICT_PACKS * PACK) == 0 or t + 1 == T:
                p1 = (t + 1 + PACK - 1) // PACK
                emit_rec(rec_next, p1)
                emit_evict(rec_next, p1)
                rec_next = p1
                while (store_next + 1) * STORE_CHUNK <= rec_next * PACK:
                    emit_store(store_next)
                    store_next += 1
        assert rec_next == n_packs
        assert store_next * STORE_CHUNK == T

    nc.compile()
    return nc


_NC_CACHE: dict[str, bass.Bass] = {}


def _get_nc() -> bass.Bass:
    if "nc" not in _NC_CACHE:
        _NC_CACHE["nc"] = build_nc()
    return _NC_CACHE["nc"]


def _prep_core(Xi: np.ndarray) -> dict[str, np.ndarray]:
    """Xi: [T, C, W] fp32 -> host-side fp16 layouts."""
    xa = np.empty((C, T, 129), dtype=np.float16)
    xa[:, :, :128] = Xi.transpose(1, 0, 2)
    xa[:, :, 128] = 1.0
    xt = np.ascontiguousarray(Xi.transpose(2, 0, 1)).astype(np.float16)
    return {
        "Xa": xa.reshape(128, T * 129),
        "Xt": xt.reshape(128, T * 128),
    }


def run(X: np.ndarray, **spmd_kwargs):
    """Shard over batch, run on 8 cores, gather.  Returns (output, results)."""
    assert X.shape == (B, T, C, W), X.shape
    nc = _get_nc()
    in_maps = [_prep_core(X[i]) for i in range(N_CORES)]
    res = run_bass_kernel_spmd(nc, in_maps, list(range(N_CORES)), **spmd_kwargs)
    out = np.stack(
        [
            res.results[i]["out"]
            .reshape(C, T, W)
            .transpose(1, 0, 2)
            .astype(np.float32)
            for i in range(N_CORES)
        ],
        axis=0,
    )
    return out, res


def kernel(X: np.ndarray) -> np.ndarray:
    out, _ = run(np.asarray(X, dtype=np.float32))
    return out
